# revision 30
# baseline (speedup 1.0000x reference)
"""Batched masked dot-product attention on 8 TRN2 NeuronCores — v2.

Problem: query/key/value [16, 2048, 64] f32, valid_len [16] i32.
out = softmax(mask(Q K^T / 8)) V, softmax over the key axis.

v2 changes vs v1 (55-63us):
  - MM2 operands swapped: V' [128k, 66] is the STATIONARY operand and the
    exp tile [128k, 512q] is the moving one, so each k-tile needs ONE MM2
    of N=512 per q-group (instead of 4 LDWEIGHTS-dominated N=66 matmuls),
    accumulating out^T [66c, 512q] in PSUM across the slot's k-tiles.
  - All input DMAs moved off the Scalar ring (sync/vector/gpsimd) so the
    ACT engine only runs the exp ACTIVATEs — it is the critical engine.
  - Output is copied PSUM->SBUF by DVE ([66, 512] f32) and DMA'd per
    (slot, q-group); host transposes/sums fragments and divides by Z.
  - Optional DVE exp offload (Schraudolph magic-constant trick) for a
    subset of (pair, qg) units to split the exp work between the ACT and
    DVE engines; controlled by DVE_EXP_FRAC.
"""

from functools import lru_cache
from itertools import product

import numpy as np
import ml_dtypes

import concourse.bass as bass  # noqa: F401
import concourse.mybir as mybir
import concourse.tile as tile
from concourse import bacc
from concourse.bass_utils import run_bass_kernel_spmd

B, L, D = 16, 2048, 64
N_CORES = 8
P = 128                 # k-tile size (partition dim)
N_KTILES = L // P       # 16
N_KBLK = N_KTILES // 2  # max packed K^T column blocks per slot
VC = 66                 # V' columns: 64 V + 1 mask + 1 pad
QG = 512                # q-group width (one PSUM bank of out^T)
N_QG = L // QG          # 4
SCALE = 1.0 / np.sqrt(np.float32(D))
F32 = mybir.dt.float32
BF16 = mybir.dt.bfloat16
FP8 = mybir.dt.float8e4
NP_BF16 = ml_dtypes.bfloat16
NP_FP8 = ml_dtypes.float8_e4m3
V8W = 80                # fp8 weight plane stride (16-byte aligned >= VC)
# ACT-flavor exps are emitted as fp8e4m3 pairs consumed by DoubleRow
# matmuls; exp is scaled by 2^-5 (bias=-5 ln 2) so the largest scaled
# score in this problem's data (8.44, batch 13) stays under e4m3's 240
# max (exp overflow -> inf -> NaN output). The scale cancels in the
# final V/Z division because every fragment of an ACT-flavor row is
# scaled identically.
EXP8_BIAS = float(-5.0 * np.log(np.float32(2.0)))
# fp8 DoubleRow MM2 measured ~241ns vs 427ns bf16, but e4m3 V'
# quantization puts ~3-6% error on peaked-softmax rows (the top
# weight's V row passes straight through) -> rel err 4.4e-2 > 2e-2.
USE_FP8_DR = False

# Schraudolph/magic-constant DVE exp: low16(bits(A*x + C)) is the bf16
# pattern of ~exp(x*SCALE) when C = 1.5*2^23 + 16256 + B_ADJ.
LOG2E = 1.4426950408889634
DVE_A = float(np.float32(128.0 * LOG2E * SCALE))
B_ADJ = 0.0
DVE_C = float(np.float32(12582912.0) + np.float32(16256.0 + B_ADJ))
# q-groups whose exp runs on DVE (must be a pure function of qg so every
# fragment of a batch row uses the same exp flavor — approx-vs-exact bias
# must not vary across the k fragments of one softmax row)
DVE_QGS = (1, 3)
# software-pipeline depth: MM2s trail their MM1/exp by this many units.
# The PE clock-gate (HAM) stays at 1.2 GHz until ~21.5us wall time no
# matter what (measured: activity does NOT un-throttle it early), so the
# early-unit MM2s are deferred much further (DELAY_COLD) than late ones
# (DELAY_WARM): during the cold phase the PE then only runs the cheap
# MM1s and stays ahead of the exp engines, and the MM2 backlog drains at
# 2.4 GHz once warm. BACKS_PER_FRONT caps the catch-up rate so queued
# MM2 bursts never starve later MM1s.
DELAY_COLD = 8
DELAY_WARM = 3
COLD_BACKS = 16   # backs with index < this get a ramped-down extra delay
BACKS_PER_FRONT = 2
BOUNDARY_EXTRA = 2
# (s,qg) output copies are deferred this many emit_backs so exps of
# later units enqueue ahead of them in the ACT/DVE FIFOs.
COPY_DEFER = 2

LAST_RESULTS = None


def _ensure_ntff_hook():
    try:
        import antenv.axon_hooks  # noqa: F401
        return
    except ImportError:
        pass
    import sys
    import types

    import antenv

    mod = types.ModuleType("antenv.axon_hooks")
    holder = [None]
    mod.set_axon_ntff_profile_hook = lambda h: holder.__setitem__(0, h)
    mod.get_axon_ntff_profile_hook = lambda: holder[0]
    sys.modules["antenv.axon_hooks"] = mod
    antenv.axon_hooks = mod
    try:
        from trn_agent_boot.trn_boot import _ntff_profile_via_ctypes

        holder[0] = _ntff_profile_via_ctypes("/opt/axon/libaxon_pjrt.so")
    except Exception:
        pass


# --------------------------------------------------------------------------
# slot-shape search (same scheme as v1: uniform per-core slot sizes)
# --------------------------------------------------------------------------

def _shape_cost(shape):
    # exp work scales with total k-tiles; prefer fewer pair-instructions too
    return sum(shape) + 0.2 * sum((n + 1) // 2 for n in shape if n)


def _solve_shape(need):
    nb = len(need)
    order = np.argsort(need)[::-1]
    sneed = tuple(int(need[i]) for i in order)

    def feasible(sizes):
        ns = len(sizes)

        @lru_cache(maxsize=None)
        def dp(i, avail):
            if i == nb:
                return ()
            n = sneed[i]
            maxc = tuple(min(avail[k], -(-n // sizes[k])) for k in range(ns))
            for combo in product(*(range(c + 1) for c in maxc)):
                cap = sum(x * s for x, s in zip(combo, sizes))
                if cap < n:
                    continue
                if any(x > 0 and cap - s >= n for x, s in zip(combo, sizes)):
                    continue  # non-minimal cover
                rest = dp(i + 1, tuple(a - x for a, x in zip(avail, combo)))
                if rest is not None:
                    return (combo,) + rest
            return None

        return dp(0, tuple(N_CORES for _ in sizes))

    pair_shape = (max(sneed[i] for i in range(N_CORES)),
                  max(sneed[i] for i in range(N_CORES, nb)), 0)

    cands = []
    for a in range(1, N_KTILES + 1):
        for b in range(0, a + 1):
            for c in range(0, b + 1):
                for dd in range(0, c + 1):
                    if (a + b + c + dd) * N_CORES < sum(sneed):
                        continue
                    cost = _shape_cost((a, b, c, dd))
                    if cost < _shape_cost(pair_shape):
                        cands.append((cost, a + b + c + dd, (a, b, c, dd)))
    cands.sort()

    for _, __, shape in cands:
        sizes = tuple(s for s in shape if s > 0)
        sol = feasible(sizes)
        if sol is None:
            continue
        return sizes, sol, order
    sizes = tuple(s for s in pair_shape if s > 0)
    return sizes, feasible(sizes), order


def _assign(need, sizes, sol, order):
    """-> frags[core][slot] = (batch, t0, length) or None."""
    ns = len(sizes)
    free = [list(range(N_CORES)) for _ in range(ns)]
    frags = [[None] * ns for _ in range(N_CORES)]
    for rank, combo in enumerate(sol):
        b = int(order[rank])
        t0, rem = 0, int(need[b])
        picks = []
        for k in range(ns):
            picks.extend([k] * combo[k])
        picks.sort(key=lambda k: -sizes[k])
        for k in picks:
            core = free[k].pop()
            ln = min(rem, sizes[k])
            frags[core][k] = (b, t0, ln)
            t0 += ln
            rem -= ln
        assert rem == 0
    return frags


def _build(sizes):
    """Build the SPMD kernel for per-core slot k-tile counts `sizes`."""
    ns = len(sizes)
    nc = bacc.Bacc("TRN2", target_bir_lowering=False, debug=False,
                   num_devices=N_CORES)
    fc_d = nc.dram_tensor("fc", [ns, P, P + QG], BF16,
                          kind="ExternalInput").ap()
    qt_d = nc.dram_tensor("qt", [ns, P, L], BF16, kind="ExternalInput").ap()
    kt_d = nc.dram_tensor("kt", [ns, P, N_KBLK * P], BF16,
                          kind="ExternalInput").ap()
    vp_d = nc.dram_tensor("vp", [ns, P, N_KTILES * VC], BF16,
                          kind="ExternalInput").ap()
    v8_d = nc.dram_tensor("v8", [ns, P, N_KBLK * 2 * V8W], FP8,
                          kind="ExternalInput").ap()
    ot_d = nc.dram_tensor("ot", [ns, N_QG, VC, QG], F32,
                          kind="ExternalOutput").ap()

    # Flatten the whole per-core schedule into "units": one unit = one
    # k-tile pair (or odd single) of one q-group. Adjacent units alternate
    # q-group parity so their exps land on DIFFERENT engines (ACT for even
    # q-groups, DVE magic-exp for odd) and run concurrently. Units are
    # emitted software-pipelined: MM1+exp of unit u, then MM2 of unit u-1,
    # so the PE never sits in FIFO order behind an exp it doesn't need.
    # process slots smallest-first: small slots have frequent accumulator
    # boundaries (pipeline stalls) that overlap the cold ramp / DMA lead-in,
    # leaving the largest slot's long smooth phases for the steady state
    sorder = sorted(range(ns), key=lambda s: sizes[s])
    snext = {sorder[i]: sorder[i + 1] for i in range(ns - 1)}
    units = []
    for s in sorder:
        nblk = (sizes[s] + 1) // 2
        for qp in range(N_QG // 2):
            for j in range(nblk):
                for parity in range(2):
                    units.append((s, qp, parity, j))
    # make the very last unit ACT-flavor (shorter exp + cheaper DoubleRow
    # MM2 shortens the drain tail)
    units[-1], units[-2] = units[-2], units[-1]

    with tile.TileContext(nc) as tc:
        with (
            tc.tile_pool(name="cb", bufs=1) as cbp,
            tc.tile_pool(name="io", bufs=3) as io,
            tc.tile_pool(name="pt", bufs=12) as ptp,
            tc.tile_pool(name="ot", bufs=4) as otp,
            tc.tile_pool(name="ps_s", bufs=3, space="PSUM") as pss,
            tc.tile_pool(name="ps_o", bufs=2, space="PSUM") as pso,
        ):
            slot_tiles = {}
            po_tiles = {}
            exp_tiles = {}

            # [128,1] f32 constant holding ln(0.5) for the fp8 exp bias
            cb = cbp.tile([P, 1], F32, tag="cb")
            nc.vector.memset(cb[:], EXP8_BIAS)

            def load_slot(s, first=False):
                nk = sizes[s]
                nblk = (nk + 1) // 2
                nprs = nk // 2
                qt = io.tile([P, L], BF16, tag="qt")
                kt = io.tile([P, N_KBLK * P], BF16, tag="kt")
                vp = io.tile([P, N_KTILES * VC], BF16, tag="vp")
                v8 = io.tile([P, N_KBLK * 2 * V8W], FP8, tag="v8")
                fc = io.tile([P, P + QG], BF16, tag="fc")
                if first:
                    # split the startup-critical transfers across BOTH hwdge
                    # queues (by partition half) so descriptor generation
                    # overlaps, and split qt so early q-groups unblock before
                    # the whole 384KB transfer completes (unit 1 needs
                    # qt[:, 512:1024] almost immediately after unit 0)
                    h = P // 2
                    nc.sync.dma_start(fc[:h], fc_d[s, :h])
                    nc.scalar.dma_start(fc[h:], fc_d[s, h:])
                    nc.sync.dma_start(qt[:h, QG:2 * QG],
                                      qt_d[s, :h, QG:2 * QG])
                    nc.scalar.dma_start(qt[h:, QG:2 * QG],
                                        qt_d[s, h:, QG:2 * QG])
                    nc.sync.dma_start(vp[:, : nk * VC], vp_d[s, :, : nk * VC])
                    nc.scalar.dma_start(qt[:, 2 * QG:3 * QG],
                                        qt_d[s, :, 2 * QG:3 * QG])
                    nc.sync.dma_start(qt[:, 3 * QG:], qt_d[s, :, 3 * QG:])
                    if nblk > 1:
                        nc.sync.dma_start(kt[:, P: nblk * P],
                                          kt_d[s, :, P: nblk * P])
                else:
                    # MM1 operands first (fc, kt, early q-groups) — MM2
                    # weights (vp/v8) are consumed a deferred-back later
                    nc.sync.dma_start(fc[:], fc_d[s])
                    if nblk > 1:
                        nc.sync.dma_start(kt[:, P: nblk * P],
                                          kt_d[s, :, P: nblk * P])
                    nc.sync.dma_start(qt[:, QG:2 * QG], qt_d[s, :, QG:2 * QG])
                    nc.sync.dma_start(qt[:, 2 * QG:], qt_d[s, :, 2 * QG:])
                    nc.sync.dma_start(vp[:, : nk * VC], vp_d[s, :, : nk * VC])
                if USE_FP8_DR and nprs:
                    nc.sync.dma_start(v8[:, : nprs * 2 * V8W],
                                      v8_d[s, :, : nprs * 2 * V8W])
                slot_tiles[s] = (qt, kt, vp, fc, v8)

            def tiles_of(s, j):
                nk = sizes[s]
                return [2 * j] + ([2 * j + 1] if 2 * j + 1 < nk else [])

            def emit_front(u):
                """MM1 pair + exp for unit u."""
                s, qp, parity, j = u
                qg = 2 * qp + parity
                qt, kt, vp, fc, v8 = slot_tiles[s]
                tl = tiles_of(s, j)
                w = len(tl) * QG
                ps = pss.tile([P, 2 * QG], F32, tag="ps")
                for idx, _ in enumerate(tl):
                    r0, r1 = (0, D) if idx == 0 else (D, P)
                    lhs = (fc[r0:r1, :P] if j == 0
                           else kt[r0:r1, j * P:(j + 1) * P])
                    rhs = (fc[r0:r1, P:P + QG] if qg == 0
                           else qt[r0:r1, qg * QG:(qg + 1) * QG])
                    nc.tensor.matmul(
                        ps[:, idx * QG:(idx + 1) * QG], lhs, rhs,
                        tile_position=((r0, 0) if len(tl) > 1 else None),
                    )
                if qg in DVE_QGS:
                    # f32 magic-constant exp: low 16 bits of each f32 are
                    # the bf16 pattern of ~exp(x*SCALE)
                    ptf = ptp.tile([P, 2 * QG], F32, tag="ptf")
                    nc.vector.tensor_scalar(
                        ptf[:, :w], ps[:, :w], DVE_A, DVE_C,
                        mybir.AluOpType.mult, mybir.AluOpType.add)
                    exp_tiles[u] = ("dve", ptf)
                elif USE_FP8_DR and len(tl) == 2:
                    # ACT-flavor pair: emit 0.5*exp as fp8e4m3 with the two
                    # k-tiles' values byte-interleaved, ready to be the
                    # DoubleRow moving operand
                    p8 = ptp.tile([P, 2 * QG], FP8, tag="p8")
                    src = ps[:, :2 * QG].rearrange("p (two n) -> p two n",
                                                   two=2)
                    dst = p8[:, :2 * QG].rearrange("p (n two) -> p two n",
                                                   two=2)
                    nc.scalar.activation(
                        dst, src, mybir.ActivationFunctionType.Exp,
                        bias=cb[:], scale=float(SCALE))
                    exp_tiles[u] = ("act8", p8)
                else:
                    pt = ptp.tile([P, 2 * QG], BF16, tag="pt")
                    if USE_FP8_DR:
                        nc.scalar.activation(
                            pt[:, :w], ps[:, :w],
                            mybir.ActivationFunctionType.Exp,
                            bias=cb[:], scale=float(SCALE))
                    else:
                        nc.scalar.activation(
                            pt[:, :w], ps[:, :w],
                            mybir.ActivationFunctionType.Exp,
                            scale=float(SCALE))
                    exp_tiles[u] = ("act", pt)

            pending = []  # completed (s, qg, parity, po) awaiting copy+DMA

            def flush_copies(n=None, split=False):
                """Emit copy+DMA for the oldest pending (s,qg) drains."""
                while pending and (n is None or len(pending) > n):
                    s, qg, parity, po = pending.pop(0)
                    ot = otp.tile([VC, QG], F32, tag="ot")
                    if split:
                        # halve across both engines and both HWDGE rings to
                        # shorten the drain tail
                        nc.vector.tensor_copy(ot[:, :QG // 2],
                                              po[:, :QG // 2])
                        nc.scalar.copy(ot[:, QG // 2:], po[:, QG // 2:])
                        nc.sync.dma_start(ot_d[s, qg, :, :QG // 2],
                                          ot[:, :QG // 2])
                        nc.scalar.dma_start(ot_d[s, qg, :, QG // 2:],
                                            ot[:, QG // 2:])
                    else:
                        # cross-assign: even-qg copies ride DVE, odd ride ACT
                        if parity == 0:
                            nc.vector.tensor_copy(ot[:], po[:])
                        else:
                            nc.scalar.copy(ot[:], po[:])
                        nc.sync.dma_start(ot_d[s, qg], ot[:])

            def emit_back(u):
                """MM2(s) for unit u; queue copy + DMA when its (s,qg) ends."""
                s, qp, parity, j = u
                qg = 2 * qp + parity
                nk = sizes[s]
                qt, kt, vp, fc, v8 = slot_tiles[s]
                tl = tiles_of(s, j)
                if (s, qg) not in po_tiles:
                    # the pool only has 2 po banks and 2 accumulators are
                    # live: every retired qg's copy must be emitted before
                    # this alloc can legally reuse its bank
                    flush_copies(0)
                    po = pso.tile([VC, QG], F32, tag="po")
                    po_tiles[(s, qg)] = po
                po = po_tiles[(s, qg)]
                kind, et = exp_tiles.pop(u)
                if kind == "act8":
                    # one DoubleRow matmul accumulates both k-tiles of the
                    # pair: contraction over 2*128 interleaved fp8 rows
                    lhsT = v8[:, j * 2 * V8W:(j + 1) * 2 * V8W].rearrange(
                        "p (two c) -> p two c", two=2)[:, :, :VC]
                    rhs = et[:, :2 * QG].rearrange("p (n two) -> p two n",
                                                   two=2)
                    nc.tensor.matmul(
                        po[:], lhsT, rhs,
                        start=(tl[0] == 0), stop=(tl[-1] == nk - 1),
                        perf_mode=mybir.MatmulPerfMode.DoubleRow,
                    )
                else:
                    for idx, t in enumerate(tl):
                        if kind == "dve":
                            ev = et.bitcast(BF16)
                            rhs = ev[:, 2 * idx * QG: 2 * (idx + 1) * QG: 2]
                        else:
                            rhs = et[:, idx * QG:(idx + 1) * QG]
                        nc.tensor.matmul(
                            po[:], vp[:, t * VC:(t + 1) * VC], rhs,
                            start=(t == 0), stop=(t == nk - 1),
                        )
                last_j = (nk + 1) // 2 - 1
                if j == last_j:
                    pending.append((s, qg, parity, po))
                    del po_tiles[(s, qg)]
                flush_copies(COPY_DEFER)

            # per-unit back delay: ramp from DELAY_COLD (early units, PE at
            # 1.2 GHz) down to DELAY_WARM; first unit of each (s,qp)
            # accumulation waits BOUNDARY_EXTRA longer (its po bank may
            # still be copying)
            seen_qg = set()
            delay = []
            for bi, u in enumerate(units):
                if bi < COLD_BACKS:
                    d = DELAY_COLD - (DELAY_COLD - DELAY_WARM) * bi \
                        // COLD_BACKS
                elif bi >= len(units) - 4:
                    d = 2  # drain the MM2 backlog before the final units
                else:
                    d = DELAY_WARM
                key = (u[0], 2 * u[1] + u[2])
                if key not in seen_qg:
                    seen_qg.add(key)
                    d += BOUNDARY_EXTRA
                delay.append(d)

            slot_first = {}
            for i, u in enumerate(units):
                slot_first.setdefault(u[0], i)

            load_slot(sorder[0], first=True)
            nb_done = 0
            for i, u in enumerate(units):
                nxt = snext.get(u[0])
                if nxt is not None and nxt not in slot_tiles \
                        and i >= slot_first[u[0]] + 2:
                    load_slot(nxt)  # prefetch next slot's inputs
                emit_front(u)
                nfront = 0
                while nb_done < len(units) and nb_done + delay[nb_done] <= i \
                        and nfront < BACKS_PER_FRONT:
                    emit_back(units[nb_done])
                    nb_done += 1
                    nfront += 1
            while nb_done < len(units):
                emit_back(units[nb_done])
                nb_done += 1
            flush_copies(0, split=True)
    nc.compile()
    return nc


def kernel(query, key, value, valid_len):
    global LAST_RESULTS
    query = np.asarray(query, np.float32)
    key = np.asarray(key, np.float32)
    value = np.asarray(value, np.float32)
    assert query.shape == (B, L, D) and np.shape(valid_len) == (B,)

    vl = np.clip(np.asarray(valid_len).astype(np.int64), 1, L)
    need = np.maximum(1, -(-vl // P))  # ceil(vl/128), in [1, 16]

    try:
        sizes, sol, order = _solve_shape(tuple(int(n) for n in need))
    except Exception:
        order = np.argsort(need)[::-1]
        sizes = (int(need[order[0]]), int(need[order[N_CORES]]))
        sol = tuple((1, 0) if r < N_CORES else (0, 1)
                    for r in range(B))
    frags = _assign(need, sizes, sol, order)
    ns = len(sizes)

    nc = _build(sizes)

    qts = {}
    kts = {}
    vps = {}
    for bi in range(B):
        qT = query[bi].T.astype(NP_BF16)
        qts[bi] = np.concatenate([qT, qT], axis=0)  # [128, L]
        kts[bi] = key[bi].T.astype(NP_BF16)         # [64, L]
        m = (np.arange(L) < vl[bi]).astype(np.float32)
        vprime = np.zeros((L, VC), np.float32)
        vprime[:, :D] = value[bi] * m[:, None]
        vprime[:, D] = m
        vps[bi] = vprime

    in_maps = []
    for c in range(N_CORES):
        fc = np.zeros((ns, P, P + QG), NP_BF16)
        qt = np.zeros((ns, P, L), NP_BF16)
        kt = np.zeros((ns, P, N_KBLK * P), NP_BF16)
        vp = np.zeros((ns, P, N_KTILES * VC), NP_BF16)
        v8 = np.zeros((ns, P, N_KBLK * 2 * V8W), NP_FP8)
        for s in range(ns):
            fr = frags[c][s]
            if fr is None:
                continue
            bi, t0, ln = fr
            qt[s] = qts[bi]
            kT = kts[bi]
            for u in range(0, ln, 2):
                blk = u // 2
                ta = t0 + u
                kt[s, :D, blk * P:(blk + 1) * P] = \
                    kT[:, ta * P:(ta + 1) * P]
                if u + 1 < ln:
                    kt[s, D:, blk * P:(blk + 1) * P] = \
                        kT[:, (ta + 1) * P:(ta + 2) * P]
            vfrag = vps[bi][t0 * P:(t0 + ln) * P].reshape(ln, P, VC)
            vp[s, :, :ln * VC] = vfrag.transpose(1, 0, 2).reshape(
                P, ln * VC).astype(NP_BF16)
            # fp8 DoubleRow weights: pair j holds tiles (2j, 2j+1) as two
            # separate 16B-aligned planes; tiles beyond the fragment stay
            # zero (their exp(0)=0.5 column is annihilated)
            zt = np.zeros((P, VC), np.float32)
            for j in range(sizes[s] // 2 if USE_FP8_DR else 0):
                a = vfrag[2 * j] if 2 * j < ln else zt
                b = vfrag[2 * j + 1] if 2 * j + 1 < ln else zt
                base = j * 2 * V8W
                v8[s, :, base:base + VC] = a.astype(NP_FP8)
                v8[s, :, base + V8W:base + V8W + VC] = b.astype(NP_FP8)
            fc[s, :, :P] = kt[s, :, :P]
            fc[s, :, P:] = qt[s, :, :QG]
        in_maps.append({"fc": fc, "qt": qt, "kt": kt, "vp": vp, "v8": v8})

    _ensure_ntff_hook()
    res = run_bass_kernel_spmd(nc, in_maps, core_ids=list(range(N_CORES)))
    LAST_RESULTS = res

    acc = np.zeros((B, L, VC), np.float64)
    for c in range(N_CORES):
        o = res.results[c]["ot"]  # [ns, N_QG, VC, QG]
        o = o.transpose(0, 1, 3, 2).reshape(ns, L, VC)  # [slot, q, c]
        for s in range(ns):
            fr = frags[c][s]
            if fr is None:
                continue
            acc[fr[0]] += o[s]
    out = (acc[:, :, :D] / acc[:, :, D:D + 1]).astype(np.float32)
    return out

